# revision 1
# baseline (speedup 1.0000x reference)
"""Trainium2 Bass kernel for EnhancedGatedFusion (MoE routing, top-2 of 8 experts).

Strategy: data-parallel over tokens across 8 NeuronCores. Each core gets
T=1024 tokens (full weights replicated) and computes:
  router logits (true fp32 matmul - top-2 selection is precision critical),
  top-2 softmax gate weights via dense max/mask trick,
  dense 8-expert MLP (float32r matmuls at full PE rate) accumulated in a
  transposed C^T [D, T] layout so expert bias is per-partition and no
  transpose is needed before the projection matmul,
  projection + residual + RMSNorm in token-major layout.
"""

import sys

for _p in ("/opt/trn_rl_repo",):
    if _p not in sys.path:
        sys.path.insert(0, _p)

from contextlib import ExitStack

import numpy as np

import concourse.bass as bass
import concourse.mybir as mybir
import concourse.tile as tile
from concourse import bacc
from concourse.masks import make_identity

FP32 = mybir.dt.float32
FP32R = mybir.dt.float32r
BF16 = mybir.dt.bfloat16
AX = mybir.AxisListType
ALU = mybir.AluOpType
ACTF = mybir.ActivationFunctionType

EPS = 1e-6
NEG_BIG = -1e30


def _bcast_ap(ap, nparts=128):
    """Partition-broadcast view of a DRAM AP (step-0 partition dim)."""
    return bass.AP(tensor=ap.tensor, offset=ap.offset, ap=[[0, nparts], *ap.ap])


def build_moe_nc(D, E, T, PW=256, trn_type="TRN2", expert_bf16=False):
    """Emit the per-core MoE program. Returns a compiled Bacc instance.

    D: model dim (multiple of 128); E: num experts; T: tokens per core;
    PW: weight panel width (multiple of 128, >=256 for f32r full rate on proj).
    """
    P = 128
    KO = D // P          # contraction k-tiles
    NTT = T // P         # token tiles of 128
    TOKMM = min(512, T)  # moving-operand token chunk for expert matmuls
    NTH = T // TOKMM     # token chunks
    NCP = D // PW        # weight panels (expert cols / proj cols)
    NCT = PW // P        # col-tiles of 128 per panel

    nc = bacc.Bacc(trn_type, target_bir_lowering=False, debug=False)

    xt = nc.dram_tensor("xt", [D, T], FP32, kind="ExternalInput").ap()
    xtb = (nc.dram_tensor("xtb", [D, T], BF16, kind="ExternalInput").ap()
           if expert_bf16 else None)
    xr = nc.dram_tensor("xr", [T, D], FP32, kind="ExternalInput").ap()
    router_w = nc.dram_tensor("router_w", [D, E], FP32, kind="ExternalInput").ap()
    router_b = nc.dram_tensor("router_b", [E], FP32, kind="ExternalInput").ap()
    ew_dt = BF16 if expert_bf16 else FP32
    expert_w = nc.dram_tensor("expert_w", [E, D, D], ew_dt, kind="ExternalInput").ap()
    expert_b = nc.dram_tensor("expert_b", [E, D], FP32, kind="ExternalInput").ap()
    proj_w = nc.dram_tensor("proj_w", [D, D], FP32, kind="ExternalInput").ap()
    proj_b = nc.dram_tensor("proj_b", [D], FP32, kind="ExternalInput").ap()
    norm_w = nc.dram_tensor("norm_w", [D], FP32, kind="ExternalInput").ap()
    out = nc.dram_tensor("out", [T, D], FP32, kind="ExternalOutput").ap()
    fw_dram = nc.dram_tensor("fw_scratch", [E, T], FP32).ap()

    xt_r = xt.rearrange("(ko p) t -> p ko t", p=P)
    rw_r = router_w.rearrange("(ko p) e -> p ko e", p=P)

    with tile.TileContext(nc) as tc, ExitStack() as ctx:
        v = nc.vector
        s = nc.scalar

        big = ctx.enter_context(tc.tile_pool(name="big", bufs=1))
        ct_pool = ctx.enter_context(tc.tile_pool(name="ct_pool", bufs=1))
        w_pool = ctx.enter_context(tc.tile_pool(name="w_pool", bufs=2))
        sil_pool = ctx.enter_context(tc.tile_pool(name="sil_pool", bufs=3))
        small = ctx.enter_context(tc.tile_pool(name="small", bufs=2))
        singles = ctx.enter_context(tc.tile_pool(name="singles", bufs=1))
        xres_pool = ctx.enter_context(tc.tile_pool(name="xres_pool", bufs=1 if expert_bf16 else 2))

        # ---- resident loads (small tensors first so the router's
        # weights aren't queued behind 8MiB of xt traffic) ----
        rw_sb = singles.tile([P, KO, E], FP32)
        nc.sync.dma_start(out=rw_sb, in_=rw_r)
        rb_rep = singles.tile([P, E], FP32)
        nc.sync.dma_start(out=rb_rep, in_=_bcast_ap(router_b))
        nw_rep = singles.tile([P, D], FP32)
        nc.sync.dma_start(out=nw_rep, in_=_bcast_ap(norm_w))
        if expert_bf16:
            xmm_sb = big.tile([P, KO, T], BF16, tag="big", name="xtb_sb")
            xtb_r = xtb.rearrange("(ko p) t -> p ko t", p=P)
            for ko in range(KO):
                nc.sync.dma_start(out=xmm_sb[:, ko, :], in_=xtb_r[:, ko, :])
            rxt_pool = ctx.enter_context(tc.tile_pool(name="rxt_pool", bufs=1))
        else:
            xmm_sb = big.tile([P, KO, T], FP32R, tag="big", name="xt_sb")
            for ko in range(KO):
                eng = nc.sync if ko % 2 == 0 else nc.scalar
                eng.dma_start(
                    out=xmm_sb[:, ko, :], in_=xt_r[:, ko, :].bitcast(FP32R)
                )

        identity = singles.tile([P, P], FP32)
        make_identity(nc, identity)
        eps_t = singles.tile([P, 1], FP32)
        v.memset(eps_t, EPS)

        fwT = singles.tile([E, T], FP32)  # gate weights, expert-major
        ct = ct_pool.tile([P, KO, T], FP32R)  # C^T accumulator [D, T]

        pse = tc.alloc_tile_pool(name="pse", bufs=6, space="PSUM")

        def panel_mms(eidx, cq, wp):
            tiles = []
            for c2 in range(NCT):
                for th in range(NTH):
                    ps = pse.tile([P, TOKMM], FP32, tag="ps",
                                  name=f"ps{eidx}_{cq}_{c2}_{th}")
                    for ko in range(KO):
                        nc.tensor.matmul(
                            ps,
                            lhsT=wp[:, ko, c2 * P:(c2 + 1) * P],
                            rhs=xmm_sb[:, ko, th * TOKMM:(th + 1) * TOKMM],
                            start=(ko == 0),
                            stop=(ko == KO - 1),
                        )
                    tiles.append(ps)
            return tiles

        def panel_gating(eidx, cq, tiles, fw_rep, eb_sb):
            idx = 0
            for c2 in range(NCT):
                colt = cq * NCT + c2
                for th in range(NTH):
                    ps = tiles[idx]
                    idx += 1
                    sg = sil_pool.tile([P, TOKMM], FP32, tag="sg",
                                       name=f"sg{eidx}_{cq}_{c2}_{th}")
                    s.activation(
                        sg, ps, ACTF.Sigmoid, bias=eb_sb[:, colt:colt + 1]
                    )
                    sil = sil_pool.tile([P, TOKMM], FP32, tag="sil",
                                        name=f"sil{eidx}_{cq}_{c2}_{th}")
                    v.scalar_tensor_tensor(
                        out=sil, in0=ps, scalar=eb_sb[:, colt:colt + 1],
                        in1=sg, op0=ALU.add, op1=ALU.mult,
                    )
                    ct_sl = ct[:, colt, th * TOKMM:(th + 1) * TOKMM]
                    fw_sl = fw_rep[:, th * TOKMM:(th + 1) * TOKMM]
                    if eidx == 0:
                        v.tensor_tensor(out=ct_sl, in0=sil, in1=fw_sl,
                                        op=ALU.mult)
                    else:
                        v.tensor_tensor(out=sil, in0=sil, in1=fw_sl,
                                        op=ALU.mult)
                        v.tensor_tensor(out=ct_sl, in0=ct_sl, in1=sil,
                                        op=ALU.add)

        def load_panel(eidx, cq, we_r):
            if expert_bf16:
                wp = w_pool.tile([P, KO, PW], BF16, tag="wp",
                                 name=f"wp{eidx}_{cq}")
                nc.sync.dma_start(out=wp, in_=we_r[:, :, cq * PW:(cq + 1) * PW])
            else:
                wp = w_pool.tile([P, KO, PW], FP32R, tag="wp",
                                 name=f"wp{eidx}_{cq}")
                weng = nc.sync if cq % 2 == 0 else nc.scalar
                weng.dma_start(
                    out=wp, in_=we_r[:, :, cq * PW:(cq + 1) * PW].bitcast(FP32R)
                )
            return wp

        # head start: expert 0's first panel matmuls fill the PE while xt
        # finishes loading and the router's DVE chain runs
        we0_r = expert_w[0].rearrange("(ko p) c -> p ko c", p=P)
        eb0_sb = small.tile([P, KO], FP32, name="eb0")
        nc.scalar.dma_start(
            out=eb0_sb, in_=expert_b[0].rearrange("(ko p) -> p ko", p=P)
        )
        wp00 = load_panel(0, 0, we0_r)
        head_tiles = panel_mms(0, 0, wp00)

        # ---- router + top-2 softmax gates ----
        with (
            tc.tile_pool(name="psr", bufs=1, space="PSUM") as psr,
            tc.tile_pool(name="pst", bufs=1, space="PSUM") as pst,
            tc.tile_pool(name="rsm", bufs=2) as rsm,
            tc.tile_pool(name="fwp", bufs=NTT) as fwp,
        ):
            fw_tiles = []
            for tt in range(NTT):
                if expert_bf16:
                    xtf = rxt_pool.tile([P, KO, P], FP32, tag="rxt")
                    nc.sync.dma_start(
                        out=xtf, in_=xt_r[:, :, tt * P:(tt + 1) * P]
                    )
                else:
                    xtf = xmm_sb[:, :, tt * P:(tt + 1) * P].bitcast(FP32)
                ps_l = psr.tile([P, E], FP32)
                for ko in range(KO):
                    nc.tensor.matmul(
                        ps_l,
                        lhsT=xtf[:, ko, :],
                        rhs=rw_sb[:, ko, :],
                        start=(ko == 0),
                        stop=(ko == KO - 1),
                    )
                logits = rsm.tile([P, E], FP32)
                v.tensor_tensor(out=logits, in0=ps_l, in1=rb_rep, op=ALU.add)
                m1 = rsm.tile([P, 1], FP32)
                v.tensor_reduce(m1, logits, axis=AX.X, op=ALU.max)
                mask1 = rsm.tile([P, E], FP32)
                v.tensor_scalar(mask1, logits, m1, None, op0=ALU.is_ge)
                lg2 = rsm.tile([P, E], FP32)
                v.scalar_tensor_tensor(
                    out=lg2, in0=mask1, scalar=NEG_BIG, in1=logits,
                    op0=ALU.mult, op1=ALU.add,
                )
                m2 = rsm.tile([P, 1], FP32)
                v.tensor_reduce(m2, lg2, axis=AX.X, op=ALU.max)
                mask2 = rsm.tile([P, E], FP32)
                v.tensor_scalar(mask2, lg2, m2, None, op0=ALU.is_ge)
                d21 = rsm.tile([P, 1], FP32)
                v.tensor_tensor(out=d21, in0=m2, in1=m1, op=ALU.subtract)
                e2 = rsm.tile([P, 1], FP32)
                s.activation(e2, d21, ACTF.Exp)
                den = rsm.tile([P, 1], FP32)
                v.tensor_scalar(den, e2, 1.0, None, op0=ALU.add)
                winv = rsm.tile([P, 1], FP32)
                v.reciprocal(winv, den)
                w2 = rsm.tile([P, 1], FP32)
                v.tensor_tensor(out=w2, in0=e2, in1=winv, op=ALU.mult)
                t2 = rsm.tile([P, E], FP32)
                v.tensor_scalar(t2, mask2, w2, None, op0=ALU.mult)
                fw = fwp.tile([P, E], FP32, tag="fw", name=f"fw{tt}")
                v.scalar_tensor_tensor(
                    out=fw, in0=mask1, scalar=winv, in1=t2,
                    op0=ALU.mult, op1=ALU.add,
                )
                fw_tiles.append(fw)
            for tt in range(NTT):
                ps_t = pst.tile([E, P], FP32)
                nc.tensor.transpose(ps_t, fw_tiles[tt], identity)
                v.tensor_copy(out=fwT[:, tt * P:(tt + 1) * P], in_=ps_t)
            nc.sync.dma_start(out=fw_dram, in_=fwT)

        # ---- expert phase: ct[d, t] = sum_e gate[e,t] * silu(x @ We + be)^T ----
        for e in range(E):
            fw_rep = sil_pool.tile([P, T], FP32, tag="fwrep",
                                   bufs=1 if expert_bf16 else 2,
                                   name=f"fwrep{e}")
            nc.sync.dma_start(out=fw_rep, in_=_bcast_ap(fw_dram[e]))
            if e == 0:
                eb_sb = eb0_sb
                we_r = we0_r
            else:
                eb_sb = small.tile([P, KO], FP32, name=f"eb{e}")
                nc.sync.dma_start(
                    out=eb_sb, in_=expert_b[e].rearrange("(ko p) -> p ko", p=P)
                )
                we_r = expert_w[e].rearrange("(ko p) c -> p ko c", p=P)
            for cq in range(NCP):
                if e == 0 and cq == 0:
                    tiles = head_tiles
                else:
                    wp = load_panel(e, cq, we_r)
                    tiles = panel_mms(e, cq, wp)
                panel_gating(e, cq, tiles, fw_rep, eb_sb)

        pse.release()

        # ---- projection + residual into Y (token-major), reusing xt's slot ----
        y_all = big.tile([P, NTT, D], FP32, tag="big")
        pw_r = proj_w.rearrange("(ko p) c -> p ko c", p=P)
        with (
            tc.tile_pool(name="psp", bufs=6, space="PSUM") as psp,
            tc.tile_pool(name="nsm", bufs=2) as nsm,
        ):
            HD = D // 2

            def emit_norm(tt):
                # RMS norm (in place on Y[tt]) + store, interleaved with proj
                y_t = y_all[:, tt, :]
                sq = nsm.tile([P, HD], FP32, tag="sq", bufs=1, name=f"sq{tt}")
                ssa = nsm.tile([P, 1], FP32, tag="ssa", name=f"ssa{tt}")
                ssb = nsm.tile([P, 1], FP32, tag="ssb", name=f"ssb{tt}")
                s.activation(sq, y_t[:, :HD], ACTF.Square, accum_out=ssa)
                s.activation(sq, y_t[:, HD:], ACTF.Square, accum_out=ssb)
                ssum = nsm.tile([P, 1], FP32, tag="ssum", name=f"ssum{tt}")
                v.tensor_tensor(out=ssum, in0=ssa, in1=ssb, op=ALU.add)
                rms = nsm.tile([P, 1], FP32, tag="rms", name=f"rms{tt}")
                s.activation(rms, ssum, ACTF.Sqrt, bias=eps_t, scale=1.0 / D)
                rinv = nsm.tile([P, 1], FP32, tag="rinv", name=f"rinv{tt}")
                v.reciprocal(rinv, rms)
                s.mul(y_t, y_t, rinv)
                v.tensor_tensor(out=y_t, in0=y_t, in1=nw_rep, op=ALU.mult)
                oeng = nc.sync if tt % 2 == 0 else nc.scalar
                oeng.dma_start(out=out[tt * P:(tt + 1) * P, :], in_=y_t)

            NG = min(2, NTT)
            TG = NTT // NG
            for tg, pp in [(g, p) for g in range(NG) for p in range(NCP)]:
                pwp = w_pool.tile([P, KO, PW], FP32R, tag="wp")
                nc.sync.dma_start(out=pwp, in_=pw_r[:, :, pp * PW:(pp + 1) * PW].bitcast(FP32R))
                prb = xres_pool.tile([P, PW], FP32, tag="prb", bufs=2)
                nc.scalar.dma_start(out=prb, in_=_bcast_ap(proj_b[pp * PW:(pp + 1) * PW]))
                for tt in range(tg * TG, (tg + 1) * TG):
                    ps_o = psp.tile([P, PW], FP32)
                    for ko in range(KO):
                        nc.tensor.matmul(
                            ps_o,
                            lhsT=ct[:, ko, tt * P:(tt + 1) * P],
                            rhs=pwp[:, ko, :],
                            start=(ko == 0),
                            stop=(ko == KO - 1),
                        )
                    xres = xres_pool.tile([P, PW], FP32)
                    nc.scalar.dma_start(
                        out=xres,
                        in_=xr[tt * P:(tt + 1) * P, pp * PW:(pp + 1) * PW],
                    )
                    y_sl = y_all[:, tt, pp * PW:(pp + 1) * PW]
                    v.tensor_tensor(out=y_sl, in0=ps_o, in1=prb, op=ALU.add)
                    v.tensor_tensor(out=y_sl, in0=y_sl, in1=xres, op=ALU.add)
                    if pp == NCP - 1:
                        emit_norm(tt)

    nc.compile()
    return nc


# ---- full-problem entry point ----
_B, _S, _D, _E = 4, 2048, 2048, 8
_NCORES = 8
_T = _B * _S // _NCORES

_EXPERT_BF16 = False

_nc_cache = None


def _get_nc():
    global _nc_cache
    if _nc_cache is None:
        _nc_cache = build_moe_nc(_D, _E, _T, expert_bf16=_EXPERT_BF16)
    return _nc_cache


def _make_in_maps(xf, router_w, router_b, expert_w, expert_b, proj_w, proj_b,
                  norm_w):
    if _EXPERT_BF16:
        import ml_dtypes
        expert_w_c = expert_w.astype(ml_dtypes.bfloat16)
    else:
        expert_w_c = expert_w
    in_maps = []
    for c in range(_NCORES):
        xs = xf[c * _T:(c + 1) * _T]
        xst = np.ascontiguousarray(xs.T)
        m = {
            "xt": xst,
            "xr": np.ascontiguousarray(xs),
            "router_w": router_w,
            "router_b": router_b,
            "expert_w": expert_w_c,
            "expert_b": expert_b,
            "proj_w": proj_w,
            "proj_b": proj_b,
            "norm_w": norm_w,
        }
        if _EXPERT_BF16:
            import ml_dtypes
            m["xtb"] = xst.astype(ml_dtypes.bfloat16)
        in_maps.append(m)
    return in_maps


def kernel(x, router_w, router_b, expert_w, expert_b, proj_w, proj_b, norm_w):
    from concourse import bass_utils

    x = np.asarray(x, np.float32)
    router_w = np.asarray(router_w, np.float32)
    router_b = np.asarray(router_b, np.float32)
    expert_w = np.asarray(expert_w, np.float32)
    expert_b = np.asarray(expert_b, np.float32)
    proj_w = np.asarray(proj_w, np.float32)
    proj_b = np.asarray(proj_b, np.float32)
    norm_w = np.asarray(norm_w, np.float32)

    nc = _get_nc()
    xf = x.reshape(-1, _D)
    in_maps = _make_in_maps(xf, router_w, router_b, expert_w, expert_b,
                            proj_w, proj_b, norm_w)
    res = bass_utils.run_bass_kernel_spmd(nc, in_maps, core_ids=list(range(_NCORES)))
    outs = [res.results[c]["out"] for c in range(_NCORES)]
    return np.concatenate(outs, axis=0).reshape(_B, _S, _D).astype(np.float32)



# revision 5
# speedup vs baseline: 2.1755x; 2.1755x over previous
"""Trainium2 Bass kernel for EnhancedGatedFusion (MoE routing, top-2 of 8).

Strategy: data-parallel over tokens across 8 NeuronCores, exploiting top-2
sparsity. The host computes the router (cheap: T*D*E MACs, 0.4% of FLOPs),
picks top-2 experts per token, and pre-gathers tokens into per-expert slot
segments (capacity C, padded). Each core then runs only the sparse expert
compute:

  expert matmuls over gathered slots (bf16, slot-major output):
      Yg[slot, :] = silu(xg[slot] @ W_e + b_e)   for slot in expert e's segment
  Yg rows stream to DRAM; a gpsimd indirect DMA gathers each token's two
  slot rows back (token-major), which are combined with the softmax gates:
      ct[t] = g1[t]*Yg[s1[t]] + g2[t]*Yg[s2[t]]
  ct token-tiles are PE-transposed into contraction-major ctT, then the
  dense projection + residual + RMSNorm tail runs exactly like the dense
  kernel (bf16 proj weights, biases folded into PSUM via K=1 matmuls).

This cuts expert FLOPs 8/2.67x (dense-8 -> top-2 + padding) and makes the
kernel PE-bound at ~0.5M+0.26M+0.05M PE rows vs 2.4M for dense.
"""

import sys

for _p in ("/opt/trn_rl_repo",):
    if _p not in sys.path:
        sys.path.insert(0, _p)

from contextlib import ExitStack

import numpy as np

import concourse.bass as bass
import concourse.mybir as mybir
import concourse.tile as tile
from concourse import bacc
from concourse.masks import make_identity

FP32 = mybir.dt.float32
BF16 = mybir.dt.bfloat16
INT32 = mybir.dt.int32
AX = mybir.AxisListType
ALU = mybir.AluOpType
ACTF = mybir.ActivationFunctionType

EPS = 1e-6


def _bcast_ap(ap, nparts=128):
    """Partition-broadcast view of a DRAM AP (step-0 partition dim)."""
    return bass.AP(tensor=ap.tensor, offset=ap.offset, ap=[[0, nparts], *ap.ap])


def build_sparse_moe_nc(D, E, T, C, trn_type="TRN2"):
    """Per-core sparse MoE program. C = per-expert slot capacity (mult of 128)."""
    P = 128
    KO = D // P          # contraction k-tiles (16)
    NTT = T // P         # token tiles (8)
    S = E * C            # total slots
    SPE = C // P         # slot tiles per expert
    WCH = 512            # expert weight moving chunk (psum free dim)
    NWC = D // WCH       # col chunks (4)
    PPW = 512            # proj panel width
    NPP = D // PPW

    nc = bacc.Bacc(trn_type, target_bir_lowering=False, debug=False)

    xgt = nc.dram_tensor("xgt", [D, S], BF16, kind="ExternalInput").ap()
    xr = nc.dram_tensor("xr", [T, D], FP32, kind="ExternalInput").ap()
    idx1 = nc.dram_tensor("idx1", [T], INT32, kind="ExternalInput").ap()
    idx2 = nc.dram_tensor("idx2", [T], INT32, kind="ExternalInput").ap()
    g1 = nc.dram_tensor("g1", [T], FP32, kind="ExternalInput").ap()
    g2 = nc.dram_tensor("g2", [T], FP32, kind="ExternalInput").ap()
    expert_w = nc.dram_tensor("expert_w", [E, D, D], BF16, kind="ExternalInput").ap()
    expert_b = nc.dram_tensor("expert_b", [E, D], BF16, kind="ExternalInput").ap()
    proj_w = nc.dram_tensor("proj_w", [D, D], BF16, kind="ExternalInput").ap()
    proj_b = nc.dram_tensor("proj_b", [D], BF16, kind="ExternalInput").ap()
    norm_w = nc.dram_tensor("norm_w", [D], FP32, kind="ExternalInput").ap()
    out = nc.dram_tensor("out", [T, D], FP32, kind="ExternalOutput").ap()
    yg = nc.dram_tensor("yg_scratch", [S, D], BF16).ap()

    xg_r = xgt.rearrange("(ko p) s -> p ko s", p=P)
    pw_r = proj_w.rearrange("(ko p) c -> p ko c", p=P)
    HKO = KO // 2

    with tile.TileContext(nc) as tc, ExitStack() as ctx:
        v = nc.vector
        s = nc.scalar

        singles = ctx.enter_context(tc.tile_pool(name="singles", bufs=1))
        xg_pool = ctx.enter_context(tc.tile_pool(name="xg_pool", bufs=2))
        w_pool = ctx.enter_context(tc.tile_pool(name="w_pool", bufs=2))
        eb_pool = ctx.enter_context(tc.tile_pool(name="eb_pool", bufs=2))
        sil_pool = ctx.enter_context(tc.tile_pool(name="sil_pool", bufs=3))
        comb_pool = ctx.enter_context(tc.tile_pool(name="comb_pool", bufs=2))
        ct_pool = ctx.enter_context(tc.tile_pool(name="ct_pool", bufs=1))
        y_pool = ctx.enter_context(tc.tile_pool(name="y_pool", bufs=1))
        xres_pool = ctx.enter_context(tc.tile_pool(name="xres_pool", bufs=2))

        # ---- small resident tensors ----
        ones1 = singles.tile([1, P], BF16)
        v.memset(ones1, 1.0)
        pbsb = singles.tile([1, D], BF16)
        nc.sync.dma_start(out=pbsb, in_=_bcast_ap(proj_b, 1))
        nw_rep = singles.tile([P, D], FP32)
        nc.sync.dma_start(out=nw_rep, in_=_bcast_ap(norm_w))
        idx1_sb = singles.tile([P, NTT], INT32)
        nc.sync.dma_start(out=idx1_sb, in_=idx1.rearrange("(tt p) -> p tt", p=P))
        idx2_sb = singles.tile([P, NTT], INT32)
        nc.sync.dma_start(out=idx2_sb, in_=idx2.rearrange("(tt p) -> p tt", p=P))
        g1_sb = singles.tile([P, NTT], FP32)
        nc.sync.dma_start(out=g1_sb, in_=g1.rearrange("(tt p) -> p tt", p=P))
        g2_sb = singles.tile([P, NTT], FP32)
        nc.sync.dma_start(out=g2_sb, in_=g2.rearrange("(tt p) -> p tt", p=P))
        identity = singles.tile([P, P], FP32)
        make_identity(nc, identity)
        eps_t = singles.tile([P, 1], FP32)
        v.memset(eps_t, EPS)

        # ---- expert phase: Yg[slot, :] = silu(xg[slot] @ We + be), slot-major ----
        pse = tc.alloc_tile_pool(name="pse", bufs=6, space="PSUM")
        for e in range(E):
            we_r = expert_w[e].rearrange("(ko p) c -> p ko c", p=P)
            xg_e = xg_pool.tile([P, KO, C], BF16, tag="xg", name=f"xg{e}")
            nc.sync.dma_start(out=xg_e[:, :HKO, :], in_=xg_r[:, :HKO, e * C:(e + 1) * C])
            nc.scalar.dma_start(out=xg_e[:, HKO:, :], in_=xg_r[:, HKO:, e * C:(e + 1) * C])
            ebsb = eb_pool.tile([1, D], BF16, tag="eb", name=f"eb{e}")
            nc.scalar.dma_start(out=ebsb, in_=_bcast_ap(expert_b[e], 1))
            for cq in range(NWC):
                wp = w_pool.tile([P, KO, WCH], BF16, tag="wp", name=f"wp{e}_{cq}")
                nc.sync.dma_start(out=wp[:, :HKO, :], in_=we_r[:, :HKO, cq * WCH:(cq + 1) * WCH])
                nc.scalar.dma_start(out=wp[:, HKO:, :], in_=we_r[:, HKO:, cq * WCH:(cq + 1) * WCH])
                for st in range(SPE):
                    sbase = (e * SPE + st) * P
                    ps = pse.tile([P, WCH], FP32, tag="ps", name=f"ps{e}_{cq}_{st}")
                    # bias via K=1 matmul: ps = ones^T @ eb_chunk
                    nc.tensor.matmul(
                        ps, lhsT=ones1, rhs=ebsb[:, cq * WCH:(cq + 1) * WCH],
                        start=True, stop=False, skip_group_check=True,
                    )
                    for ko in range(KO):
                        nc.tensor.matmul(
                            ps,
                            lhsT=xg_e[:, ko, st * P:(st + 1) * P],
                            rhs=wp[:, ko, :],
                            start=False, stop=(ko == KO - 1),
                            skip_group_check=True,
                        )
                    sg = sil_pool.tile([P, WCH], FP32, tag="sg", name=f"sg{e}_{cq}_{st}")
                    s.activation(sg, ps, ACTF.Sigmoid)
                    ygt = sil_pool.tile([P, WCH], BF16, tag="ygt", name=f"ygt{e}_{cq}_{st}")
                    v.tensor_tensor(out=ygt, in0=ps, in1=sg, op=ALU.mult)
                    oeng = nc.sync if (cq + st) % 2 == 0 else nc.scalar
                    oeng.dma_start(
                        out=yg[sbase:sbase + P, cq * WCH:(cq + 1) * WCH], in_=ygt
                    )
        pse.release()

        # ---- combine: ct[t] = g1*Yg[s1[t]] + g2*Yg[s2[t]], then transpose ----
        ctT = ct_pool.tile([P, KO, T], BF16)
        with tc.tile_pool(name="psT", bufs=2, space="PSUM") as psT:
            for tt in range(NTT):
                y1 = comb_pool.tile([P, D], BF16, tag="y1", name=f"y1_{tt}")
                nc.gpsimd.indirect_dma_start(
                    out=y1, out_offset=None, in_=yg,
                    in_offset=bass.IndirectOffsetOnAxis(ap=idx1_sb[:, tt:tt + 1], axis=0),
                )
                y2 = comb_pool.tile([P, D], BF16, tag="y2", name=f"y2_{tt}")
                nc.gpsimd.indirect_dma_start(
                    out=y2, out_offset=None, in_=yg,
                    in_offset=bass.IndirectOffsetOnAxis(ap=idx2_sb[:, tt:tt + 1], axis=0),
                )
                ctt = comb_pool.tile([P, D], FP32, tag="ctt", name=f"ctt{tt}")
                v.tensor_scalar(ctt, y1, g1_sb[:, tt:tt + 1], None, op0=ALU.mult)
                v.scalar_tensor_tensor(
                    out=ctt, in0=y2, scalar=g2_sb[:, tt:tt + 1], in1=ctt,
                    op0=ALU.mult, op1=ALU.add,
                )
                for ko in range(KO):
                    pst = psT.tile([P, P], FP32, tag="pst", name=f"pst{tt}_{ko}")
                    nc.tensor.transpose(pst, ctt[:, ko * P:(ko + 1) * P], identity)
                    v.tensor_copy(out=ctT[:, ko, tt * P:(tt + 1) * P], in_=pst)

        # ---- projection + residual + RMSNorm (token-major) ----
        # Two token groups so the y buffer is half-size (SBUF); proj panels
        # are reloaded per group (+8.4MB DMA, hidden under compute).
        NG = 2
        TG = NTT // NG
        with (
            tc.tile_pool(name="psp", bufs=6, space="PSUM") as psp,
            tc.tile_pool(name="nsm", bufs=2) as nsm,
        ):
            HD = D // 2

            def emit_norm(y_all, tt_local, tt):
                y_t = y_all[:, tt_local, :]
                sq = nsm.tile([P, HD], FP32, tag="sq", bufs=1, name=f"sq{tt}")
                ssa = nsm.tile([P, 1], FP32, tag="ssa", name=f"ssa{tt}")
                ssb = nsm.tile([P, 1], FP32, tag="ssb", name=f"ssb{tt}")
                s.activation(sq, y_t[:, :HD], ACTF.Square, accum_out=ssa)
                s.activation(sq, y_t[:, HD:], ACTF.Square, accum_out=ssb)
                ssum = nsm.tile([P, 1], FP32, tag="ssum", name=f"ssum{tt}")
                v.tensor_tensor(out=ssum, in0=ssa, in1=ssb, op=ALU.add)
                rms = nsm.tile([P, 1], FP32, tag="rms", name=f"rms{tt}")
                s.activation(rms, ssum, ACTF.Sqrt, bias=eps_t, scale=1.0 / D)
                rinv = nsm.tile([P, 1], FP32, tag="rinv", name=f"rinv{tt}")
                v.reciprocal(rinv, rms)
                s.mul(y_t, y_t, rinv)
                v.tensor_tensor(out=y_t, in0=y_t, in1=nw_rep, op=ALU.mult)
                oeng = nc.sync if tt % 2 == 0 else nc.scalar
                oeng.dma_start(out=out[tt * P:(tt + 1) * P, :], in_=y_t)

            for tg in range(NG):
                y_all = y_pool.tile([P, TG, D], FP32, tag="y", name=f"y_all{tg}")
                for pp in range(NPP):
                    pwp = w_pool.tile([P, KO, PPW], BF16, tag="wp", name=f"pwp{tg}_{pp}")
                    nc.sync.dma_start(out=pwp[:, :HKO, :], in_=pw_r[:, :HKO, pp * PPW:(pp + 1) * PPW])
                    nc.scalar.dma_start(out=pwp[:, HKO:, :], in_=pw_r[:, HKO:, pp * PPW:(pp + 1) * PPW])
                    for tt_local in range(TG):
                        tt = tg * TG + tt_local
                        ps_o = psp.tile([P, PPW], FP32, tag="ps", name=f"pso{pp}_{tt}")
                        nc.tensor.matmul(
                            ps_o, lhsT=ones1, rhs=pbsb[:, pp * PPW:(pp + 1) * PPW],
                            start=True, stop=False, skip_group_check=True,
                        )
                        for ko in range(KO):
                            nc.tensor.matmul(
                                ps_o,
                                lhsT=ctT[:, ko, tt * P:(tt + 1) * P],
                                rhs=pwp[:, ko, :],
                                start=False, stop=(ko == KO - 1),
                                skip_group_check=True,
                            )
                        xres = xres_pool.tile([P, PPW], FP32, tag="xres", name=f"xres{pp}_{tt}")
                        nc.scalar.dma_start(
                            out=xres, in_=xr[tt * P:(tt + 1) * P, pp * PPW:(pp + 1) * PPW]
                        )
                        y_sl = y_all[:, tt_local, pp * PPW:(pp + 1) * PPW]
                        v.tensor_tensor(out=y_sl, in0=ps_o, in1=xres, op=ALU.add)
                        if pp == NPP - 1:
                            emit_norm(y_all, tt_local, tt)

    nc.compile()
    return nc


# ---- full-problem entry point ----
_B, _SEQ, _D, _E = 4, 2048, 2048, 8
_NCORES = 8
_T = _B * _SEQ // _NCORES

_nc_cache = {}


def _get_nc(C):
    if C not in _nc_cache:
        _nc_cache[C] = build_sparse_moe_nc(_D, _E, _T, C)
    return _nc_cache[C]


def _route(xf, router_w, router_b):
    """Host router: fp32 logits (matching reference precision), top-2, gates."""
    logits = xf @ router_w + router_b                       # [Tn, E] fp32
    order = np.argsort(-logits, axis=1, kind="stable")      # jax top_k tie-break
    i1 = order[:, 0]
    i2 = order[:, 1]
    tn = np.arange(logits.shape[0])
    l1 = logits[tn, i1].astype(np.float64)
    l2 = logits[tn, i2].astype(np.float64)
    e2 = np.exp(l2 - l1)
    den = 1.0 + e2
    w1 = (1.0 / den).astype(np.float32)
    w2 = (e2 / den).astype(np.float32)
    return i1, i2, w1, w2


def _make_in_maps(xf, router_w, router_b, expert_w, expert_b, proj_w, proj_b,
                  norm_w):
    import ml_dtypes

    i1, i2, w1, w2 = _route(xf, router_w, router_b)

    counts = np.zeros((_NCORES, _E), np.int64)
    for c in range(_NCORES):
        sl = slice(c * _T, (c + 1) * _T)
        for e in range(_E):
            counts[c, e] = ((i1[sl] == e) | (i2[sl] == e)).sum()
    C = max(128, int(np.ceil(counts.max() / 128)) * 128)
    S = _E * C

    ew_b = np.ascontiguousarray(expert_w.astype(ml_dtypes.bfloat16))
    eb_b = np.ascontiguousarray(expert_b.astype(ml_dtypes.bfloat16))
    pw_b = np.ascontiguousarray(proj_w.astype(ml_dtypes.bfloat16))
    pb_b = np.ascontiguousarray(proj_b.astype(ml_dtypes.bfloat16))
    nw_f = np.ascontiguousarray(norm_w.astype(np.float32))

    in_maps = []
    for c in range(_NCORES):
        sl = slice(c * _T, (c + 1) * _T)
        xc = xf[sl]
        i1c, i2c, w1c, w2c = i1[sl], i2[sl], w1[sl], w2[sl]
        slot_tokens = np.zeros(S, np.int64)   # pad slots point at token 0 (unused)
        idx1c = np.zeros(_T, np.int32)
        idx2c = np.zeros(_T, np.int32)
        for e in range(_E):
            toks = np.nonzero((i1c == e) | (i2c == e))[0]
            base = e * C
            slot_tokens[base:base + len(toks)] = toks
            is1 = i1c[toks] == e
            idx1c[toks[is1]] = base + np.nonzero(is1)[0]
            idx2c[toks[~is1]] = base + np.nonzero(~is1)[0]
        xg = xc[slot_tokens]                                  # [S, D] fp32
        xgt = np.ascontiguousarray(xg.T.astype(ml_dtypes.bfloat16))  # [D, S]
        in_maps.append({
            "xgt": xgt,
            "xr": np.ascontiguousarray(xc),
            "idx1": idx1c,
            "idx2": idx2c,
            "g1": np.ascontiguousarray(w1c),
            "g2": np.ascontiguousarray(w2c),
            "expert_w": ew_b,
            "expert_b": eb_b,
            "proj_w": pw_b,
            "proj_b": pb_b,
            "norm_w": nw_f,
        })
    return in_maps, C


def kernel(x, router_w, router_b, expert_w, expert_b, proj_w, proj_b, norm_w):
    from concourse import bass_utils

    x = np.asarray(x, np.float32)
    router_w = np.asarray(router_w, np.float32)
    router_b = np.asarray(router_b, np.float32)
    expert_w = np.asarray(expert_w, np.float32)
    expert_b = np.asarray(expert_b, np.float32)
    proj_w = np.asarray(proj_w, np.float32)
    proj_b = np.asarray(proj_b, np.float32)
    norm_w = np.asarray(norm_w, np.float32)

    xf = x.reshape(-1, _D)
    in_maps, C = _make_in_maps(xf, router_w, router_b, expert_w, expert_b,
                               proj_w, proj_b, norm_w)
    nc = _get_nc(C)
    res = bass_utils.run_bass_kernel_spmd(nc, in_maps, core_ids=list(range(_NCORES)))
    outs = [res.results[c]["out"] for c in range(_NCORES)]
    return np.concatenate(outs, axis=0).reshape(_B, _SEQ, _D).astype(np.float32)


# revision 12
# speedup vs baseline: 2.4693x; 1.1351x over previous
"""Trainium2 Bass kernel for EnhancedGatedFusion (MoE routing, top-2 of 8).

Strategy: data-parallel over tokens across 8 NeuronCores, exploiting top-2
sparsity. The host computes the router (cheap: T*D*E MACs, 0.4% of FLOPs),
picks top-2 experts per token, and pre-gathers tokens into per-expert slot
segments (capacity C, padded). Each core then runs only the sparse expert
compute:

  expert matmuls over gathered slots (bf16, slot-major output):
      Yg[slot, :] = silu(xg[slot] @ W_e + b_e)   for slot in expert e's segment
  Yg rows stream to DRAM; a gpsimd indirect DMA gathers each token's two
  slot rows back (token-major), which are combined with the softmax gates:
      ct[t] = g1[t]*Yg[s1[t]] + g2[t]*Yg[s2[t]]
  ct token-tiles are PE-transposed into contraction-major ctT, then the
  dense projection + residual + RMSNorm tail runs exactly like the dense
  kernel (bf16 proj weights, biases folded into PSUM via K=1 matmuls).

This cuts expert FLOPs 8/2.67x (dense-8 -> top-2 + padding) and makes the
kernel PE-bound at ~0.5M+0.26M+0.05M PE rows vs 2.4M for dense.
"""

import sys

for _p in ("/opt/trn_rl_repo",):
    if _p not in sys.path:
        sys.path.insert(0, _p)

from contextlib import ExitStack

import numpy as np

import concourse.bass as bass
import concourse.mybir as mybir
import concourse.tile as tile
from concourse import bacc
from concourse.masks import make_identity

FP32 = mybir.dt.float32
BF16 = mybir.dt.bfloat16
INT32 = mybir.dt.int32
AX = mybir.AxisListType
ALU = mybir.AluOpType
ACTF = mybir.ActivationFunctionType

EPS = 1e-6


def _bcast_ap(ap, nparts=128):
    """Partition-broadcast view of a DRAM AP (step-0 partition dim)."""
    return bass.AP(tensor=ap.tensor, offset=ap.offset, ap=[[0, nparts], *ap.ap])


def build_sparse_moe_nc(D, E, T, C, trn_type="TRN2"):
    """Per-core sparse MoE program. C = per-expert slot capacity (mult of 128)."""
    P = 128
    KO = D // P          # contraction k-tiles (16)
    NTT = T // P         # token tiles (8)
    S = E * C            # total slots
    SPE = C // P         # slot tiles per expert
    WCH = 512            # expert weight moving chunk (psum free dim)
    NWC = D // WCH       # col chunks (4)
    PPW = 512            # proj panel width
    NPP = D // PPW

    nc = bacc.Bacc(trn_type, target_bir_lowering=False, debug=False)

    xgt = nc.dram_tensor("xgt", [D, S], BF16, kind="ExternalInput").ap()
    xr = nc.dram_tensor("xr", [T, D], FP32, kind="ExternalInput").ap()
    # per-slot scatter targets: token row if this slot is the token's
    # stream-1 (top1) / stream-2 (top2) contribution, else T (out of bounds,
    # silently dropped). gs = the token's gate weight for this slot.
    sc1 = nc.dram_tensor("sc1", [S], INT32, kind="ExternalInput").ap()
    sc2 = nc.dram_tensor("sc2", [S], INT32, kind="ExternalInput").ap()
    gs = nc.dram_tensor("gs", [S], FP32, kind="ExternalInput").ap()
    expert_w = nc.dram_tensor("expert_w", [E, D, D], BF16, kind="ExternalInput").ap()
    expert_b = nc.dram_tensor("expert_b", [E, D], BF16, kind="ExternalInput").ap()
    proj_w = nc.dram_tensor("proj_w", [D, D], BF16, kind="ExternalInput").ap()
    proj_b = nc.dram_tensor("proj_b", [D], BF16, kind="ExternalInput").ap()
    norm_w = nc.dram_tensor("norm_w", [D], FP32, kind="ExternalInput").ap()
    out = nc.dram_tensor("out", [T, D], FP32, kind="ExternalOutput").ap()
    c1 = nc.dram_tensor("c1_scratch", [T, D], BF16).ap()
    c2 = nc.dram_tensor("c2_scratch", [T, D], BF16).ap()

    xg_r = xgt.rearrange("(ko p) s -> p ko s", p=P)
    pw_r = proj_w.rearrange("(ko p) c -> p ko c", p=P)
    HKO = KO // 2

    with tile.TileContext(nc) as tc, ExitStack() as ctx:
        v = nc.vector
        s = nc.scalar

        singles = ctx.enter_context(tc.tile_pool(name="singles", bufs=1))
        xg_pool = ctx.enter_context(tc.tile_pool(name="xg_pool", bufs=2))
        w_pool = ctx.enter_context(tc.tile_pool(name="w_pool", bufs=2))
        eb_pool = ctx.enter_context(tc.tile_pool(name="eb_pool", bufs=2))
        sil_pool = ctx.enter_context(tc.tile_pool(name="sil_pool", bufs=3))
        comb_pool = ctx.enter_context(tc.tile_pool(name="comb_pool", bufs=2))
        ct_pool = ctx.enter_context(tc.tile_pool(name="ct_pool", bufs=1))
        y_pool = ctx.enter_context(tc.tile_pool(name="y_pool", bufs=1))
        xres_pool = ctx.enter_context(tc.tile_pool(name="xres_pool", bufs=2))

        # ---- small resident tensors ----
        ones1 = singles.tile([1, P], BF16)
        v.memset(ones1, 1.0)
        pbsb = singles.tile([1, D], BF16)
        nc.sync.dma_start(out=pbsb, in_=_bcast_ap(proj_b, 1))
        nw_rep = singles.tile([P, D], FP32)
        nc.sync.dma_start(out=nw_rep, in_=_bcast_ap(norm_w))
        NST = S // P
        sc1_sb = singles.tile([P, NST], INT32)
        nc.sync.dma_start(out=sc1_sb, in_=sc1.rearrange("(n p) -> p n", p=P))
        sc2_sb = singles.tile([P, NST], INT32)
        nc.sync.dma_start(out=sc2_sb, in_=sc2.rearrange("(n p) -> p n", p=P))
        gs_sb = singles.tile([P, NST], FP32)
        nc.sync.dma_start(out=gs_sb, in_=gs.rearrange("(n p) -> p n", p=P))
        identity = singles.tile([P, P], FP32)
        make_identity(nc, identity)
        eps_t = singles.tile([P, 1], FP32)
        v.memset(eps_t, EPS)

        # ---- expert phase: Yg[slot, :] = silu(xg[slot] @ We + be), slot-major ----
        pse = tc.alloc_tile_pool(name="pse", bufs=6, space="PSUM")
        for e in range(E):
            we_r = expert_w[e].rearrange("(ko p) c -> p ko c", p=P)
            xg_e = xg_pool.tile([P, KO, C], BF16, tag="xg", name=f"xg{e}")
            nc.sync.dma_start(out=xg_e[:, :HKO, :], in_=xg_r[:, :HKO, e * C:(e + 1) * C])
            nc.scalar.dma_start(out=xg_e[:, HKO:, :], in_=xg_r[:, HKO:, e * C:(e + 1) * C])
            ebsb = eb_pool.tile([1, D], BF16, tag="eb", name=f"eb{e}")
            nc.scalar.dma_start(out=ebsb, in_=_bcast_ap(expert_b[e], 1))
            yge = sil_pool.tile([P, SPE, D], BF16, tag="yge", bufs=2, name=f"yge{e}")
            for cq in range(NWC):
                wp = w_pool.tile([P, KO, WCH], BF16, tag="wp", name=f"wp{e}_{cq}")
                nc.sync.dma_start(out=wp[:, :HKO, :], in_=we_r[:, :HKO, cq * WCH:(cq + 1) * WCH])
                nc.scalar.dma_start(out=wp[:, HKO:, :], in_=we_r[:, HKO:, cq * WCH:(cq + 1) * WCH])
                for st in range(SPE):
                    stile = e * SPE + st
                    ps = pse.tile([P, WCH], FP32, tag="ps", name=f"ps{e}_{cq}_{st}")
                    # bias via K=1 matmul: ps = ones^T @ eb_chunk
                    nc.tensor.matmul(
                        ps, lhsT=ones1, rhs=ebsb[:, cq * WCH:(cq + 1) * WCH],
                        start=True, stop=False, skip_group_check=True,
                    )
                    for ko in range(KO):
                        nc.tensor.matmul(
                            ps,
                            lhsT=xg_e[:, ko, st * P:(st + 1) * P],
                            rhs=wp[:, ko, :],
                            start=False, stop=(ko == KO - 1),
                            skip_group_check=True,
                        )
                    sg = sil_pool.tile([P, WCH], FP32, tag="sg", name=f"sg{e}_{cq}_{st}")
                    s.activation(sg, ps, ACTF.Sigmoid)
                    # gated silu: (ps * gate_slot) * sigmoid(ps), bf16 out
                    v.scalar_tensor_tensor(
                        out=yge[:, st, cq * WCH:(cq + 1) * WCH],
                        in0=ps, scalar=gs_sb[:, stile:stile + 1], in1=sg,
                        op0=ALU.mult, op1=ALU.mult,
                    )
            for st in range(SPE):
                stile = e * SPE + st
                nc.gpsimd.indirect_dma_start(
                    out=c1, out_offset=bass.IndirectOffsetOnAxis(
                        ap=sc1_sb[:, stile:stile + 1], axis=0),
                    in_=yge[:, st, :], in_offset=None,
                    bounds_check=T - 1, oob_is_err=False,
                )
                nc.gpsimd.indirect_dma_start(
                    out=c2, out_offset=bass.IndirectOffsetOnAxis(
                        ap=sc2_sb[:, stile:stile + 1], axis=0),
                    in_=yge[:, st, :], in_offset=None,
                    bounds_check=T - 1, oob_is_err=False,
                )
        pse.release()

        # ---- combine + projection, interleaved in two token groups ----
        # combine: ct[t] = g1*Yg[s1[t]] + g2*Yg[s2[t]] (row gather via gpsimd
        # indirect DMA), PE-transposed into contraction-major ctT (bf16).
        # proj group tg only needs ctT token tiles of that group, so the PE
        # stream is: transposes(g0) -> proj(g0) -> transposes(g1) -> proj(g1),
        # letting group-1 gathers/DVE run under group-0 proj matmuls.
        ctT = ct_pool.tile([P, KO, T], BF16)
        id_bf = singles.tile([P, P], BF16)
        v.tensor_copy(out=id_bf, in_=identity)
        NG = 2
        TG = NTT // NG
        KOC = 4                     # ko per psum->sbuf copy batch
        with (
            tc.tile_pool(name="psT", bufs=2, space="PSUM") as psT,
            tc.tile_pool(name="psp", bufs=6, space="PSUM") as psp,
            tc.tile_pool(name="nsm", bufs=2) as nsm,
        ):
            HD = D // 2

            def combine_tt(tt):
                y1 = comb_pool.tile([P, D], BF16, tag="y1", name=f"y1_{tt}")
                nc.sync.dma_start(out=y1, in_=c1[tt * P:(tt + 1) * P, :])
                y2 = comb_pool.tile([P, D], BF16, tag="y2", name=f"y2_{tt}")
                nc.scalar.dma_start(out=y2, in_=c2[tt * P:(tt + 1) * P, :])
                ctt = comb_pool.tile([P, D], BF16, tag="ctt", name=f"ctt{tt}")
                for half, eng in ((0, v), (1, nc.gpsimd)):
                    csl = slice(half * HD, (half + 1) * HD)
                    eng.tensor_tensor(
                        out=ctt[:, csl], in0=y1[:, csl], in1=y2[:, csl],
                        op=ALU.add,
                    )
                for kb in range(KO // KOC):
                    pst = psT.tile([P, KOC * P], BF16, tag="pst",
                                   name=f"pst{tt}_{kb}")
                    for kk in range(KOC):
                        ko = kb * KOC + kk
                        nc.tensor.transpose(
                            pst[:, kk * P:(kk + 1) * P],
                            ctt[:, ko * P:(ko + 1) * P], id_bf,
                        )
                    dst = ctT[:, kb * KOC:(kb + 1) * KOC, tt * P:(tt + 1) * P]
                    src = pst.rearrange("p (k c) -> p k c", k=KOC)
                    if kb % 2 == 0:
                        v.tensor_copy(out=dst, in_=src)
                    else:
                        s.activation(dst, src, ACTF.Copy)

            def emit_norm(y_all, tt_local, tt):
                y_t = y_all[:, tt_local, :]
                sq = nsm.tile([P, HD], FP32, tag="sq", bufs=1, name=f"sq{tt}")
                ssa = nsm.tile([P, 1], FP32, tag="ssa", name=f"ssa{tt}")
                ssb = nsm.tile([P, 1], FP32, tag="ssb", name=f"ssb{tt}")
                s.activation(sq, y_t[:, :HD], ACTF.Square, accum_out=ssa)
                s.activation(sq, y_t[:, HD:], ACTF.Square, accum_out=ssb)
                ssum = nsm.tile([P, 1], FP32, tag="ssum", name=f"ssum{tt}")
                v.tensor_tensor(out=ssum, in0=ssa, in1=ssb, op=ALU.add)
                rms = nsm.tile([P, 1], FP32, tag="rms", name=f"rms{tt}")
                s.activation(rms, ssum, ACTF.Sqrt, bias=eps_t, scale=1.0 / D)
                rinv = nsm.tile([P, 1], FP32, tag="rinv", name=f"rinv{tt}")
                v.reciprocal(rinv, rms)
                s.mul(y_t, y_t, rinv)
                v.tensor_tensor(out=y_t, in0=y_t, in1=nw_rep, op=ALU.mult)
                oeng = nc.sync if tt % 2 == 0 else nc.scalar
                oeng.dma_start(out=out[tt * P:(tt + 1) * P, :], in_=y_t)

            for tg in range(NG):
                for tt in range(tg * TG, (tg + 1) * TG):
                    combine_tt(tt)
                y_all = y_pool.tile([P, TG, D], FP32, tag="y", name=f"y_all{tg}")
                for pp in range(NPP):
                    pwp = w_pool.tile([P, KO, PPW], BF16, tag="wp", name=f"pwp{tg}_{pp}")
                    nc.sync.dma_start(out=pwp[:, :HKO, :], in_=pw_r[:, :HKO, pp * PPW:(pp + 1) * PPW])
                    nc.scalar.dma_start(out=pwp[:, HKO:, :], in_=pw_r[:, HKO:, pp * PPW:(pp + 1) * PPW])
                    for tt_local in range(TG):
                        tt = tg * TG + tt_local
                        ps_o = psp.tile([P, PPW], FP32, tag="ps", name=f"pso{pp}_{tt}")
                        nc.tensor.matmul(
                            ps_o, lhsT=ones1, rhs=pbsb[:, pp * PPW:(pp + 1) * PPW],
                            start=True, stop=False, skip_group_check=True,
                        )
                        for ko in range(KO):
                            nc.tensor.matmul(
                                ps_o,
                                lhsT=ctT[:, ko, tt * P:(tt + 1) * P],
                                rhs=pwp[:, ko, :],
                                start=False, stop=(ko == KO - 1),
                                skip_group_check=True,
                            )
                        xres = xres_pool.tile([P, PPW], FP32, tag="xres", name=f"xres{pp}_{tt}")
                        nc.scalar.dma_start(
                            out=xres, in_=xr[tt * P:(tt + 1) * P, pp * PPW:(pp + 1) * PPW]
                        )
                        y_sl = y_all[:, tt_local, pp * PPW:(pp + 1) * PPW]
                        v.tensor_tensor(out=y_sl, in0=ps_o, in1=xres, op=ALU.add)
                        if pp == NPP - 1:
                            emit_norm(y_all, tt_local, tt)

    nc.compile()
    return nc


# ---- full-problem entry point ----
_B, _SEQ, _D, _E = 4, 2048, 2048, 8
_NCORES = 8
_T = _B * _SEQ // _NCORES

_nc_cache = {}


def _get_nc(C):
    if C not in _nc_cache:
        _nc_cache[C] = build_sparse_moe_nc(_D, _E, _T, C)
    return _nc_cache[C]


def _route(xf, router_w, router_b):
    """Host router: fp32 logits (matching reference precision), top-2, gates."""
    logits = xf @ router_w + router_b                       # [Tn, E] fp32
    order = np.argsort(-logits, axis=1, kind="stable")      # jax top_k tie-break
    i1 = order[:, 0]
    i2 = order[:, 1]
    tn = np.arange(logits.shape[0])
    l1 = logits[tn, i1].astype(np.float64)
    l2 = logits[tn, i2].astype(np.float64)
    e2 = np.exp(l2 - l1)
    den = 1.0 + e2
    w1 = (1.0 / den).astype(np.float32)
    w2 = (e2 / den).astype(np.float32)
    return i1, i2, w1, w2


def _make_in_maps(xf, router_w, router_b, expert_w, expert_b, proj_w, proj_b,
                  norm_w):
    import ml_dtypes

    i1, i2, w1, w2 = _route(xf, router_w, router_b)

    counts = np.zeros((_NCORES, _E), np.int64)
    for c in range(_NCORES):
        sl = slice(c * _T, (c + 1) * _T)
        for e in range(_E):
            counts[c, e] = ((i1[sl] == e) | (i2[sl] == e)).sum()
    C = max(128, int(np.ceil(counts.max() / 128)) * 128)
    S = _E * C

    ew_b = np.ascontiguousarray(expert_w.astype(ml_dtypes.bfloat16))
    eb_b = np.ascontiguousarray(expert_b.astype(ml_dtypes.bfloat16))
    pw_b = np.ascontiguousarray(proj_w.astype(ml_dtypes.bfloat16))
    pb_b = np.ascontiguousarray(proj_b.astype(ml_dtypes.bfloat16))
    nw_f = np.ascontiguousarray(norm_w.astype(np.float32))

    in_maps = []
    for c in range(_NCORES):
        sl = slice(c * _T, (c + 1) * _T)
        xc = xf[sl]
        i1c, i2c, w1c, w2c = i1[sl], i2[sl], w1[sl], w2[sl]
        slot_tokens = np.zeros(S, np.int64)   # pad slots point at token 0 (unused)
        sc1c = np.full(S, _T, np.int32)       # T = out-of-bounds sentinel
        sc2c = np.full(S, _T, np.int32)
        gsc = np.zeros(S, np.float32)
        for e in range(_E):
            toks = np.nonzero((i1c == e) | (i2c == e))[0]
            base = e * C
            slot_tokens[base:base + len(toks)] = toks
            slots = base + np.arange(len(toks))
            is1 = i1c[toks] == e
            sc1c[slots[is1]] = toks[is1]
            sc2c[slots[~is1]] = toks[~is1]
            gsc[slots[is1]] = w1c[toks[is1]]
            gsc[slots[~is1]] = w2c[toks[~is1]]
        xg = xc[slot_tokens]                                  # [S, D] fp32
        xgt = np.ascontiguousarray(xg.T.astype(ml_dtypes.bfloat16))  # [D, S]
        in_maps.append({
            "xgt": xgt,
            "xr": np.ascontiguousarray(xc),
            "sc1": sc1c,
            "sc2": sc2c,
            "gs": gsc,
            "expert_w": ew_b,
            "expert_b": eb_b,
            "proj_w": pw_b,
            "proj_b": pb_b,
            "norm_w": nw_f,
        })
    return in_maps, C


def kernel(x, router_w, router_b, expert_w, expert_b, proj_w, proj_b, norm_w):
    from concourse import bass_utils

    x = np.asarray(x, np.float32)
    router_w = np.asarray(router_w, np.float32)
    router_b = np.asarray(router_b, np.float32)
    expert_w = np.asarray(expert_w, np.float32)
    expert_b = np.asarray(expert_b, np.float32)
    proj_w = np.asarray(proj_w, np.float32)
    proj_b = np.asarray(proj_b, np.float32)
    norm_w = np.asarray(norm_w, np.float32)

    xf = x.reshape(-1, _D)
    in_maps, C = _make_in_maps(xf, router_w, router_b, expert_w, expert_b,
                               proj_w, proj_b, norm_w)
    nc = _get_nc(C)
    res = bass_utils.run_bass_kernel_spmd(nc, in_maps, core_ids=list(range(_NCORES)))
    outs = [res.results[c]["out"] for c in range(_NCORES)]
    return np.concatenate(outs, axis=0).reshape(_B, _SEQ, _D).astype(np.float32)
